# revision 22
# baseline (speedup 1.0000x reference)
"""CenterNet decode + pseudo-NMS + top-K for Trainium2 (8 NeuronCores).

Observation: the reference only returns results for batch element 0
(`topk_scores[0]`, `topk_clses[0]`, and boxes gathered with `topk_inds[0]`),
so only `cls_pred[0]` / `txty_pred[0]` / `twth_pred[0]` influence the output.
Working on raw logits (sigmoid is strictly monotone) keeps ordering and the
5x5-peak test identical while avoiding any dense transcendentals.

Device (classes of batch 0 sharded 10-per-core across 8 cores, ~2.6 MB/core —
the memory-bound part):
  - stream the 10 class heatmaps [256,256] into SBUF as [128, 512] each
    (partition p holds image rows 2p, 2p+1), transfers split across the two
    HWDGE rings (SP + Activation) and pipelined with compute
  - 2x8 block-max pool (DVE tensor_reduce, 16:1 reduction) -> cx [128, 320]
  - ship the pooled grid back (160 KB/core, a 16x data reduction)

Host epilogue (numpy, O(candidates)):
  - take the globally largest block maxima; for each candidate block, only
    an aligned-2x2-sub-block argmax can survive the reference's 5x5 peak
    test (two surviving peaks are >= 3 apart in Chebyshev distance), so 4
    exact peak tests per block recover every peak it contains
  - stop once 100 peaks are verified AND every unexamined block's max is
    strictly below the 100th peak (provably exact; escalates the candidate
    count, up to a full scan, otherwise)
  - sigmoid + box decode only at the 100 winning positions
"""

import numpy as np

NCORES = 8
CPC = 10  # classes per core
H = W = 256
BW_ = 8  # block width (block = 2 rows x 8 cols)
NBLK = W // BW_  # 32 blocks per class per partition-row
CXW = CPC * NBLK  # 320
TOPK = 100
STRIDE = 4

_CACHE = {}


def _build_bass():
    if "nc" in _CACHE:
        return _CACHE["nc"]
    import concourse.bass as bass
    import concourse.mybir as mybir

    # Bass.__init__ ends with an all-engine barrier. This kernel never
    # touches PE/const-APs and every cross-engine dependency in the body is
    # explicitly semaphore-guarded, so the init barrier is safely elidable
    # (the Block-exit barrier still synchronizes everything at the end).
    # disable_frame_to_traceback keeps source paths out of the BIR so the
    # NEFF compile cache hits regardless of where this file lives
    _orig_aeb = bass.Bass.all_engine_barrier
    bass.Bass.all_engine_barrier = lambda self, **kw: None
    try:
        nc = bass.Bass(trn_type="TRN2", disable_frame_to_traceback=True)
    finally:
        bass.Bass.all_engine_barrier = _orig_aeb

    x = nc.dram_tensor("x", [CPC, H, W], mybir.dt.float32, kind="ExternalInput")
    # output: the full 2x8 block-max grid; host selects candidates from it
    ocx = nc.dram_tensor("ocx", [128, CXW], mybir.dt.float32, kind="ExternalOutput")

    xt = nc.alloc_sbuf_tensor("xt", [128, CPC * 512], mybir.dt.float32)
    cx = nc.alloc_sbuf_tensor("cx", [128, CXW], mybir.dt.float32)

    # input plan: (ring, class_lo, class_hi) — small first chunks for a fast
    # pipeline start, 2-class chunks in the middle for bandwidth, small last
    # chunks so the final reduce isn't gated on a large transfer
    PLAN = [
        (0, 0, 1),
        (1, 1, 2),
        (0, 2, 4),
        (1, 4, 6),
        (0, 6, 8),
        (1, 8, 9),
        (1, 9, 10),
    ]

    # one completion sem per input DMA (a shared cumulative sem would be
    # unsound: 16 SDMA engines complete independently, so sem >= 16*g does
    # not imply the g-th transfer fully landed while later ones are queued)
    gsem = [nc.alloc_semaphore(f"gsem{i}") for i in range(len(PLAN))]
    vsem = nc.alloc_semaphore("vsem")
    osem = nc.alloc_semaphore("osem")
    cls_sem = {}  # class -> transfer idx whose sem gates it
    for i, (_, c0, c1) in enumerate(PLAN):
        for c in range(c0, c1):
            cls_sem[c] = i

    def dma_in(eng, i):
        _, c0, c1 = PLAN[i]
        eng.dma_start(
            out=xt[:, c0 * 512 : c1 * 512].rearrange(
                "p (c a w) -> p c a w", c=c1 - c0, a=2
            ),
            in_=x[c0:c1].rearrange("c (p a) w -> p c a w", a=2),
        ).then_inc(gsem[i], 16)

    with nc.Block(no_gpsimd_drain=True) as block:

        @block.sync
        def _(sync):
            for i, (ring, _, _) in enumerate(PLAN):
                if ring == 0:
                    dma_in(sync, i)
            sync.wait_ge(vsem, 1)
            sync.dma_start(out=ocx[:], in_=cx[:]).then_inc(osem, 16)

        @block.scalar
        def _(scalar):
            for i, (ring, _, _) in enumerate(PLAN):
                if ring == 1:
                    dma_in(scalar, i)

        @block.vector
        def _(vector):
            waited = set()
            for c in range(CPC):
                if cls_sem[c] not in waited:
                    waited.add(cls_sem[c])
                    vector.wait_ge(gsem[cls_sem[c]], 16)
                # 2x8 block max: [p, j, a, b] -> [p, j]
                vector.reduce_max(
                    out=cx[:, c * NBLK : (c + 1) * NBLK],
                    in_=xt[:, c * 512 : (c + 1) * 512].rearrange(
                        "p (a j b) -> p j a b", a=2, b=BW_
                    ),
                    axis=mybir.AxisListType.XY,
                )
            # flush the DVE pipe before the output DMA reads cx
            vector.drain().then_inc(vsem, 1)

    # wait for the output DMA's completion receipt after the block-exit
    # barrier, so the barrier overlaps the DMA flight instead of serializing
    nc.sync.wait_ge(osem, 16)

    _CACHE["nc"] = nc
    return nc


def _run_device(cls0, trace=False, **trace_kwargs):
    """cls0: np.float32 [80,256,256] -> (cxall [8,128,320] f32, results)."""
    from concourse.bass_utils import run_bass_kernel_spmd

    nc = _build_bass()
    in_maps = [
        {"x": np.ascontiguousarray(cls0[c * CPC : (c + 1) * CPC])}
        for c in range(NCORES)
    ]
    res = run_bass_kernel_spmd(
        nc, in_maps, core_ids=list(range(NCORES)), trace=trace, **trace_kwargs
    )
    cxall = np.stack([r["ocx"] for r in res.results])
    return cxall, res


def _verify_blocks(cls0, q):
    """q: flat candidate indices into cxall [8,128,320]. Returns verified
    peaks (value, cls, y, x) found in those 2x8 blocks."""
    core = q // (128 * CXW)
    p = (q // CXW) % 128
    f = q % CXW
    cls = core * CPC + f // NBLK
    j = f % NBLK

    blk = cls0[
        cls[:, None, None],
        (2 * p)[:, None, None] + np.arange(2)[None, :, None],
        (BW_ * j)[:, None, None] + np.arange(BW_)[None, None, :],
    ]  # [N, 2, 8]
    N = blk.shape[0]
    # 4 aligned 2x2 sub-blocks; only a sub-block argmax can be a 5x5 peak
    sub = blk.reshape(N, 2, 4, 2).transpose(0, 2, 1, 3).reshape(N, 4, 4)
    pos = sub.argmax(axis=2)  # [N, 4] in 0..3
    v = np.take_along_axis(sub, pos[:, :, None], axis=2)[:, :, 0]
    y = 2 * p[:, None] + pos // 2
    x = BW_ * j[:, None] + 2 * np.arange(4)[None, :] + pos % 2
    cls4 = np.broadcast_to(cls[:, None], (N, 4))
    v, y, x, cls4 = v.ravel(), y.ravel(), x.ravel(), cls4.ravel()

    # exact 5x5 peak test (index clipping == -inf padding under max)
    d = np.arange(-2, 3)
    yy = np.clip(y[:, None] + d[None, :], 0, H - 1)
    xx = np.clip(x[:, None] + d[None, :], 0, W - 1)
    win = cls0[cls4[:, None, None], yy[:, :, None], xx[:, None, :]]
    keep = win.max(axis=(1, 2)) == v
    return v[keep], cls4[keep], y[keep], x[keep]


def _postprocess(cls0, txty0, twth0, cxall):
    flat = cxall.reshape(-1)
    ncand = 2048
    while True:
        if ncand >= flat.size:
            order = np.argsort(-flat, kind="stable")
            bound = -np.inf
        else:
            part = np.argpartition(-flat, ncand)[:ncand]
            order = part[np.argsort(-flat[part], kind="stable")]
            bound = flat[order[-1]]  # unexamined block maxima are <= bound
        v, cc, yy, xx = _verify_blocks(cls0, order)
        if v.size >= TOPK:
            # reference tie-break: lax.top_k is stable over index order, so
            # equal scores order by ascending class, then spatial position
            top = np.lexsort((yy * W + xx, cc, -v))[:TOPK]
            if bound < v[top[-1]]:
                break
        if ncand >= flat.size:
            if v.size < TOPK:  # fewer than 100 peaks exist; not reachable
                return _reference_numpy(cls0, txty0, twth0)
            break  # full scan: exact by construction
        ncand *= 8
    v, cc, yy, xx = v[top], cc[top], yy[top], xx[top]

    scores = (1.0 / (1.0 + np.exp(-v))).astype(np.float32)
    clses = cc.astype(np.int32)

    sig = lambda a: (1.0 / (1.0 + np.exp(-a.astype(np.float32)))).astype(np.float32)
    cxp = (sig(txty0[0, yy, xx]) + xx.astype(np.float32)) * STRIDE
    cyp = (sig(txty0[1, yy, xx]) + yy.astype(np.float32)) * STRIDE
    wp = np.exp(twth0[0, yy, xx].astype(np.float32)) * STRIDE
    hp = np.exp(twth0[1, yy, xx].astype(np.float32)) * STRIDE
    scale = np.float32(H * STRIDE)
    bbox = (
        np.stack([cxp - wp * 0.5, cyp - hp * 0.5, cxp + wp * 0.5, cyp + hp * 0.5], -1)
        / scale
    )
    bbox = np.clip(bbox, 0.0, 1.0).astype(np.float32)
    return bbox, scores, clses


def _reference_numpy(cls0, txty0, twth0):
    """Exact host-only implementation (device-failure safety net)."""
    prob = (1.0 / (1.0 + np.exp(-cls0.astype(np.float64)))).astype(np.float32)
    pad = np.full((prob.shape[0], H + 4, W + 4), -np.inf, np.float32)
    pad[:, 2:-2, 2:-2] = prob
    hmax = prob.copy()
    for ddy in range(5):
        for ddx in range(5):
            np.maximum(hmax, pad[:, ddy : ddy + H, ddx : ddx + W], out=hmax)
    masked = prob * (hmax == prob)
    C = masked.shape[0]
    flat = masked.reshape(C, H * W)
    i1 = np.argsort(-flat, axis=1, kind="stable")[:, :TOPK]
    s1 = np.take_along_axis(flat, i1, axis=1)
    s1f = s1.reshape(-1)
    i2 = np.argsort(-s1f, kind="stable")[:TOPK]
    clses = (i2 // TOPK).astype(np.int32)
    inds = i1.reshape(-1)[i2]
    yy, xx = inds // W, inds % W
    scores = s1f[i2].astype(np.float32)
    sig = lambda a: (1.0 / (1.0 + np.exp(-a.astype(np.float32)))).astype(np.float32)
    cxp = (sig(txty0[0, yy, xx]) + xx.astype(np.float32)) * STRIDE
    cyp = (sig(txty0[1, yy, xx]) + yy.astype(np.float32)) * STRIDE
    wp = np.exp(twth0[0, yy, xx].astype(np.float32)) * STRIDE
    hp = np.exp(twth0[1, yy, xx].astype(np.float32)) * STRIDE
    scale = np.float32(H * STRIDE)
    bbox = (
        np.stack([cxp - wp * 0.5, cyp - hp * 0.5, cxp + wp * 0.5, cyp + hp * 0.5], -1)
        / scale
    )
    return np.clip(bbox, 0.0, 1.0).astype(np.float32), scores, clses


def kernel(cls_pred, txty_pred, twth_pred):
    cls0 = np.ascontiguousarray(np.asarray(cls_pred[0], dtype=np.float32))
    txty0 = np.asarray(txty_pred[0], dtype=np.float32)
    twth0 = np.asarray(twth_pred[0], dtype=np.float32)
    # the axon-tunneled device occasionally reports NRT_EXEC_UNIT_UNRECOVERABLE
    # if a previous process tore down mid-flight; a backend reset + retry
    # recovers it, and the exact host fallback guarantees a correct answer
    for attempt in range(3):
        try:
            cxall, _ = _run_device(cls0)
            return _postprocess(cls0, txty0, twth0, cxall)
        except Exception:  # pragma: no cover - device-flake path
            if attempt == 2:
                break
            import time

            time.sleep(2.0)
            try:
                import jax.extend.backend

                jax.extend.backend.clear_backends()
            except Exception:
                pass
    return _reference_numpy(cls0, txty0, twth0)


# revision 23
# speedup vs baseline: 1.0389x; 1.0389x over previous
"""CenterNet decode + pseudo-NMS + top-K for Trainium2 (8 NeuronCores).

Observation: the reference only returns results for batch element 0
(`topk_scores[0]`, `topk_clses[0]`, and boxes gathered with `topk_inds[0]`),
so only `cls_pred[0]` / `txty_pred[0]` / `twth_pred[0]` influence the output.
Working on raw logits (sigmoid is strictly monotone) keeps ordering and the
5x5-peak test identical while avoiding any dense transcendentals.

Device (classes of batch 0 sharded 10-per-core across 8 cores, ~2.6 MB/core —
the memory-bound part):
  - stream the 10 class heatmaps [256,256] into SBUF as [128, 512] each
    (partition p holds image rows 2p, 2p+1), transfers split across the two
    HWDGE rings (SP + Activation) and pipelined with compute
  - 2x8 block-max pool (DVE tensor_reduce, 16:1 reduction) -> cx [128, 320]
  - ship the pooled grid back (160 KB/core, a 16x data reduction)

Host epilogue (numpy, O(candidates)):
  - take the globally largest block maxima; for each candidate block, only
    an aligned-2x2-sub-block argmax can survive the reference's 5x5 peak
    test (two surviving peaks are >= 3 apart in Chebyshev distance), so 4
    exact peak tests per block recover every peak it contains
  - stop once 100 peaks are verified AND every unexamined block's max is
    strictly below the 100th peak (provably exact; escalates the candidate
    count, up to a full scan, otherwise)
  - sigmoid + box decode only at the 100 winning positions
"""

import numpy as np

NCORES = 8
CPC = 10  # classes per core
H = W = 256
BW_ = 8  # block width (block = 2 rows x 8 cols)
NBLK = W // BW_  # 32 blocks per class per partition-row
CXW = CPC * NBLK  # 320
TOPK = 100
STRIDE = 4

_CACHE = {}


def _build_bass():
    if "nc" in _CACHE:
        return _CACHE["nc"]
    import concourse.bass as bass
    import concourse.mybir as mybir

    # Bass.__init__ ends with an all-engine barrier. This kernel never
    # touches PE/const-APs and every cross-engine dependency in the body is
    # explicitly semaphore-guarded, so the init barrier is safely elidable
    # (the Block-exit barrier still synchronizes everything at the end).
    # disable_frame_to_traceback keeps source paths out of the BIR so the
    # NEFF compile cache hits regardless of where this file lives
    _orig_aeb = bass.Bass.all_engine_barrier
    bass.Bass.all_engine_barrier = lambda self, **kw: None
    try:
        nc = bass.Bass(trn_type="TRN2", disable_frame_to_traceback=True)
    finally:
        bass.Bass.all_engine_barrier = _orig_aeb

    x = nc.dram_tensor("x", [CPC, H, W], mybir.dt.float32, kind="ExternalInput")
    # output: the full 2x8 block-max grid; host selects candidates from it
    ocx = nc.dram_tensor("ocx", [128, CXW], mybir.dt.float32, kind="ExternalOutput")

    xt = nc.alloc_sbuf_tensor("xt", [128, CPC * 512], mybir.dt.float32)
    cx = nc.alloc_sbuf_tensor("cx", [128, CXW], mybir.dt.float32)

    # input plan: (ring, class_lo, class_hi) — small first chunks for a fast
    # pipeline start, 2-class chunks in the middle for bandwidth, small last
    # chunks so the final reduce isn't gated on a large transfer
    PLAN = [
        (0, 0, 1),
        (1, 1, 2),
        (0, 2, 4),
        (1, 4, 6),
        (0, 6, 8),
        (1, 8, 9),
        (1, 9, 10),
    ]

    # one completion sem per input DMA (a shared cumulative sem would be
    # unsound: 16 SDMA engines complete independently, so sem >= 16*g does
    # not imply the g-th transfer fully landed while later ones are queued)
    gsem = [nc.alloc_semaphore(f"gsem{i}") for i in range(len(PLAN))]
    vsem = nc.alloc_semaphore("vsem")
    osem = nc.alloc_semaphore("osem")
    cls_sem = {}  # class -> transfer idx whose sem gates it
    for i, (_, c0, c1) in enumerate(PLAN):
        for c in range(c0, c1):
            cls_sem[c] = i

    def dma_in(eng, i):
        _, c0, c1 = PLAN[i]
        eng.dma_start(
            out=xt[:, c0 * 512 : c1 * 512].rearrange(
                "p (c a w) -> p c a w", c=c1 - c0, a=2
            ),
            in_=x[c0:c1].rearrange("c (p a) w -> p c a w", a=2),
        ).then_inc(gsem[i], 16)

    with nc.Block() as block:

        @block.sync
        def _(sync):
            for i, (ring, _, _) in enumerate(PLAN):
                if ring == 0:
                    dma_in(sync, i)
            sync.wait_ge(vsem, 1)
            sync.dma_start(out=ocx[:], in_=cx[:]).then_inc(osem, 16)

        @block.scalar
        def _(scalar):
            for i, (ring, _, _) in enumerate(PLAN):
                if ring == 1:
                    dma_in(scalar, i)

        @block.vector
        def _(vector):
            waited = set()
            for c in range(CPC):
                if cls_sem[c] not in waited:
                    waited.add(cls_sem[c])
                    vector.wait_ge(gsem[cls_sem[c]], 16)
                # 2x8 block max: [p, j, a, b] -> [p, j]
                vector.reduce_max(
                    out=cx[:, c * NBLK : (c + 1) * NBLK],
                    in_=xt[:, c * 512 : (c + 1) * 512].rearrange(
                        "p (a j b) -> p j a b", a=2, b=BW_
                    ),
                    axis=mybir.AxisListType.XY,
                )
            # flush the DVE pipe before the output DMA reads cx
            vector.drain().then_inc(vsem, 1)

    # wait for the output DMA's completion receipt after the block-exit
    # barrier, so the barrier overlaps the DMA flight instead of serializing
    nc.sync.wait_ge(osem, 16)

    _CACHE["nc"] = nc
    return nc


def _run_device(cls0, trace=False, **trace_kwargs):
    """cls0: np.float32 [80,256,256] -> (cxall [8,128,320] f32, results)."""
    from concourse.bass_utils import run_bass_kernel_spmd

    nc = _build_bass()
    in_maps = [
        {"x": np.ascontiguousarray(cls0[c * CPC : (c + 1) * CPC])}
        for c in range(NCORES)
    ]
    res = run_bass_kernel_spmd(
        nc, in_maps, core_ids=list(range(NCORES)), trace=trace, **trace_kwargs
    )
    cxall = np.stack([r["ocx"] for r in res.results])
    return cxall, res


def _verify_blocks(cls0, q):
    """q: flat candidate indices into cxall [8,128,320]. Returns verified
    peaks (value, cls, y, x) found in those 2x8 blocks."""
    core = q // (128 * CXW)
    p = (q // CXW) % 128
    f = q % CXW
    cls = core * CPC + f // NBLK
    j = f % NBLK

    blk = cls0[
        cls[:, None, None],
        (2 * p)[:, None, None] + np.arange(2)[None, :, None],
        (BW_ * j)[:, None, None] + np.arange(BW_)[None, None, :],
    ]  # [N, 2, 8]
    N = blk.shape[0]
    # 4 aligned 2x2 sub-blocks; only a sub-block argmax can be a 5x5 peak
    sub = blk.reshape(N, 2, 4, 2).transpose(0, 2, 1, 3).reshape(N, 4, 4)
    pos = sub.argmax(axis=2)  # [N, 4] in 0..3
    v = np.take_along_axis(sub, pos[:, :, None], axis=2)[:, :, 0]
    y = 2 * p[:, None] + pos // 2
    x = BW_ * j[:, None] + 2 * np.arange(4)[None, :] + pos % 2
    cls4 = np.broadcast_to(cls[:, None], (N, 4))
    v, y, x, cls4 = v.ravel(), y.ravel(), x.ravel(), cls4.ravel()

    # exact 5x5 peak test (index clipping == -inf padding under max)
    d = np.arange(-2, 3)
    yy = np.clip(y[:, None] + d[None, :], 0, H - 1)
    xx = np.clip(x[:, None] + d[None, :], 0, W - 1)
    win = cls0[cls4[:, None, None], yy[:, :, None], xx[:, None, :]]
    keep = win.max(axis=(1, 2)) == v
    return v[keep], cls4[keep], y[keep], x[keep]


def _postprocess(cls0, txty0, twth0, cxall):
    flat = cxall.reshape(-1)
    ncand = 2048
    while True:
        if ncand >= flat.size:
            order = np.argsort(-flat, kind="stable")
            bound = -np.inf
        else:
            part = np.argpartition(-flat, ncand)[:ncand]
            order = part[np.argsort(-flat[part], kind="stable")]
            bound = flat[order[-1]]  # unexamined block maxima are <= bound
        v, cc, yy, xx = _verify_blocks(cls0, order)
        if v.size >= TOPK:
            # reference tie-break: lax.top_k is stable over index order, so
            # equal scores order by ascending class, then spatial position
            top = np.lexsort((yy * W + xx, cc, -v))[:TOPK]
            if bound < v[top[-1]]:
                break
        if ncand >= flat.size:
            if v.size < TOPK:  # fewer than 100 peaks exist; not reachable
                return _reference_numpy(cls0, txty0, twth0)
            break  # full scan: exact by construction
        ncand *= 8
    v, cc, yy, xx = v[top], cc[top], yy[top], xx[top]

    scores = (1.0 / (1.0 + np.exp(-v))).astype(np.float32)
    clses = cc.astype(np.int32)

    sig = lambda a: (1.0 / (1.0 + np.exp(-a.astype(np.float32)))).astype(np.float32)
    cxp = (sig(txty0[0, yy, xx]) + xx.astype(np.float32)) * STRIDE
    cyp = (sig(txty0[1, yy, xx]) + yy.astype(np.float32)) * STRIDE
    wp = np.exp(twth0[0, yy, xx].astype(np.float32)) * STRIDE
    hp = np.exp(twth0[1, yy, xx].astype(np.float32)) * STRIDE
    scale = np.float32(H * STRIDE)
    bbox = (
        np.stack([cxp - wp * 0.5, cyp - hp * 0.5, cxp + wp * 0.5, cyp + hp * 0.5], -1)
        / scale
    )
    bbox = np.clip(bbox, 0.0, 1.0).astype(np.float32)
    return bbox, scores, clses


def _reference_numpy(cls0, txty0, twth0):
    """Exact host-only implementation (device-failure safety net)."""
    prob = (1.0 / (1.0 + np.exp(-cls0.astype(np.float64)))).astype(np.float32)
    pad = np.full((prob.shape[0], H + 4, W + 4), -np.inf, np.float32)
    pad[:, 2:-2, 2:-2] = prob
    hmax = prob.copy()
    for ddy in range(5):
        for ddx in range(5):
            np.maximum(hmax, pad[:, ddy : ddy + H, ddx : ddx + W], out=hmax)
    masked = prob * (hmax == prob)
    C = masked.shape[0]
    flat = masked.reshape(C, H * W)
    i1 = np.argsort(-flat, axis=1, kind="stable")[:, :TOPK]
    s1 = np.take_along_axis(flat, i1, axis=1)
    s1f = s1.reshape(-1)
    i2 = np.argsort(-s1f, kind="stable")[:TOPK]
    clses = (i2 // TOPK).astype(np.int32)
    inds = i1.reshape(-1)[i2]
    yy, xx = inds // W, inds % W
    scores = s1f[i2].astype(np.float32)
    sig = lambda a: (1.0 / (1.0 + np.exp(-a.astype(np.float32)))).astype(np.float32)
    cxp = (sig(txty0[0, yy, xx]) + xx.astype(np.float32)) * STRIDE
    cyp = (sig(txty0[1, yy, xx]) + yy.astype(np.float32)) * STRIDE
    wp = np.exp(twth0[0, yy, xx].astype(np.float32)) * STRIDE
    hp = np.exp(twth0[1, yy, xx].astype(np.float32)) * STRIDE
    scale = np.float32(H * STRIDE)
    bbox = (
        np.stack([cxp - wp * 0.5, cyp - hp * 0.5, cxp + wp * 0.5, cyp + hp * 0.5], -1)
        / scale
    )
    return np.clip(bbox, 0.0, 1.0).astype(np.float32), scores, clses


def kernel(cls_pred, txty_pred, twth_pred):
    cls0 = np.ascontiguousarray(np.asarray(cls_pred[0], dtype=np.float32))
    txty0 = np.asarray(txty_pred[0], dtype=np.float32)
    twth0 = np.asarray(twth_pred[0], dtype=np.float32)
    # the axon-tunneled device occasionally reports NRT_EXEC_UNIT_UNRECOVERABLE
    # if a previous process tore down mid-flight; a backend reset + retry
    # recovers it, and the exact host fallback guarantees a correct answer
    for attempt in range(3):
        try:
            cxall, _ = _run_device(cls0)
            return _postprocess(cls0, txty0, twth0, cxall)
        except Exception:  # pragma: no cover - device-flake path
            if attempt == 2:
                break
            import time

            time.sleep(2.0)
            try:
                import jax.extend.backend

                jax.extend.backend.clear_backends()
            except Exception:
                pass
    return _reference_numpy(cls0, txty0, twth0)


# revision 25
# speedup vs baseline: 1.1078x; 1.0664x over previous
"""CenterNet decode + pseudo-NMS + top-K for Trainium2 (8 NeuronCores).

Observation: the reference only returns results for batch element 0
(`topk_scores[0]`, `topk_clses[0]`, and boxes gathered with `topk_inds[0]`),
so only `cls_pred[0]` / `txty_pred[0]` / `twth_pred[0]` influence the output.
Working on raw logits (sigmoid is strictly monotone) keeps ordering and the
5x5-peak test identical while avoiding any dense transcendentals.

Device (classes of batch 0 sharded 10-per-core across 8 cores, ~2.6 MB/core —
the memory-bound part):
  - stream the 10 class heatmaps [256,256] into SBUF as [128, 512] each
    (partition p holds image rows 2p, 2p+1), transfers split across the two
    HWDGE rings (SP + Activation) and pipelined with compute
  - 2x8 block-max pool (DVE tensor_reduce, 16:1 reduction) -> cx [128, 320]
  - ship the pooled grid back (160 KB/core, a 16x data reduction)

Host epilogue (numpy, O(candidates)):
  - take the globally largest block maxima; for each candidate block, only
    an aligned-2x2-sub-block argmax can survive the reference's 5x5 peak
    test (two surviving peaks are >= 3 apart in Chebyshev distance), so 4
    exact peak tests per block recover every peak it contains
  - stop once 100 peaks are verified AND every unexamined block's max is
    strictly below the 100th peak (provably exact; escalates the candidate
    count, up to a full scan, otherwise)
  - sigmoid + box decode only at the 100 winning positions
"""

import numpy as np

NCORES = 8
CPC = 10  # classes per core
H = W = 256
BW_ = 8  # block width (block = 2 rows x 8 cols)
NBLK = W // BW_  # 32 blocks per class per partition-row
CXW = CPC * NBLK  # 320
TOPK = 100
STRIDE = 4

_CACHE = {}


def _build_bass():
    if "nc" in _CACHE:
        return _CACHE["nc"]
    import concourse.bass as bass
    import concourse.mybir as mybir

    # Bass.__init__ ends with an all-engine barrier. This kernel never
    # touches PE/const-APs and every cross-engine dependency in the body is
    # explicitly semaphore-guarded, so the init barrier is safely elidable
    # (the Block-exit barrier still synchronizes everything at the end).
    # disable_frame_to_traceback keeps source paths out of the BIR so the
    # NEFF compile cache hits regardless of where this file lives
    _orig_aeb = bass.Bass.all_engine_barrier
    bass.Bass.all_engine_barrier = lambda self, **kw: None
    try:
        nc = bass.Bass(trn_type="TRN2", disable_frame_to_traceback=True)
    finally:
        bass.Bass.all_engine_barrier = _orig_aeb

    x = nc.dram_tensor("x", [CPC, H, W], mybir.dt.float32, kind="ExternalInput")
    # output: the full 2x8 block-max grid; host selects candidates from it
    ocx = nc.dram_tensor("ocx", [128, CXW], mybir.dt.float32, kind="ExternalOutput")

    xt = nc.alloc_sbuf_tensor("xt", [128, CPC * 512], mybir.dt.float32)
    cx = nc.alloc_sbuf_tensor("cx", [128, CXW], mybir.dt.float32)

    # input plan: (ring, class_lo, class_hi) — small first chunks for a fast
    # pipeline start, 2-class chunks in the middle for bandwidth, small last
    # chunks so the final reduce isn't gated on a large transfer
    PLAN = [
        (0, 0, 1),
        (1, 1, 2),
        (0, 2, 4),
        (1, 4, 6),
        (0, 6, 8),
        (1, 8, 9),
        (1, 9, 10),
    ]

    # one completion sem per input DMA (a shared cumulative sem would be
    # unsound: 16 SDMA engines complete independently, so sem >= 16*g does
    # not imply the g-th transfer fully landed while later ones are queued)
    gsem = [nc.alloc_semaphore(f"gsem{i}") for i in range(len(PLAN))]
    vsem = nc.alloc_semaphore("vsem")
    osem = nc.alloc_semaphore("osem")
    cls_sem = {}  # class -> transfer idx whose sem gates it
    for i, (_, c0, c1) in enumerate(PLAN):
        for c in range(c0, c1):
            cls_sem[c] = i

    def dma_in(eng, i):
        _, c0, c1 = PLAN[i]
        eng.dma_start(
            out=xt[:, c0 * 512 : c1 * 512].rearrange(
                "p (c a w) -> p c a w", c=c1 - c0, a=2
            ),
            in_=x[c0:c1].rearrange("c (p a) w -> p c a w", a=2),
        ).then_inc(gsem[i], 16)

    with nc.Block() as block:

        @block.sync
        def _(sync):
            for i, (ring, _, _) in enumerate(PLAN):
                if ring == 0:
                    dma_in(sync, i)
            sync.wait_ge(vsem, 1)
            sync.dma_start(out=ocx[:], in_=cx[:]).then_inc(osem, 16)

        @block.scalar
        def _(scalar):
            for i, (ring, _, _) in enumerate(PLAN):
                if ring == 1:
                    dma_in(scalar, i)

        @block.vector
        def _(vector):
            # one reduce per arrived transfer (fewer per-op fixed overheads)
            for i, (_, c0, c1) in enumerate(PLAN):
                vector.wait_ge(gsem[i], 16)
                # 2x8 block max: [p, j, a, b] -> [p, j]
                vector.reduce_max(
                    out=cx[:, c0 * NBLK : c1 * NBLK],
                    in_=xt[:, c0 * 512 : c1 * 512].rearrange(
                        "p (c a j b) -> p c j a b", c=c1 - c0, a=2, b=BW_
                    ),
                    axis=mybir.AxisListType.XY,
                )
            # flush the DVE pipe before the output DMA reads cx
            vector.drain().then_inc(vsem, 1)

    # wait for the output DMA's completion receipt after the block-exit
    # barrier, so the barrier overlaps the DMA flight instead of serializing
    nc.sync.wait_ge(osem, 16)

    _CACHE["nc"] = nc
    return nc


def _run_device(cls0, trace=False, **trace_kwargs):
    """cls0: np.float32 [80,256,256] -> (cxall [8,128,320] f32, results)."""
    from concourse.bass_utils import run_bass_kernel_spmd

    nc = _build_bass()
    in_maps = [
        {"x": np.ascontiguousarray(cls0[c * CPC : (c + 1) * CPC])}
        for c in range(NCORES)
    ]
    res = run_bass_kernel_spmd(
        nc, in_maps, core_ids=list(range(NCORES)), trace=trace, **trace_kwargs
    )
    cxall = np.stack([r["ocx"] for r in res.results])
    return cxall, res


def _verify_blocks(cls0, q):
    """q: flat candidate indices into cxall [8,128,320]. Returns verified
    peaks (value, cls, y, x) found in those 2x8 blocks."""
    core = q // (128 * CXW)
    p = (q // CXW) % 128
    f = q % CXW
    cls = core * CPC + f // NBLK
    j = f % NBLK

    blk = cls0[
        cls[:, None, None],
        (2 * p)[:, None, None] + np.arange(2)[None, :, None],
        (BW_ * j)[:, None, None] + np.arange(BW_)[None, None, :],
    ]  # [N, 2, 8]
    N = blk.shape[0]
    # 4 aligned 2x2 sub-blocks; only a sub-block argmax can be a 5x5 peak
    sub = blk.reshape(N, 2, 4, 2).transpose(0, 2, 1, 3).reshape(N, 4, 4)
    pos = sub.argmax(axis=2)  # [N, 4] in 0..3
    v = np.take_along_axis(sub, pos[:, :, None], axis=2)[:, :, 0]
    y = 2 * p[:, None] + pos // 2
    x = BW_ * j[:, None] + 2 * np.arange(4)[None, :] + pos % 2
    cls4 = np.broadcast_to(cls[:, None], (N, 4))
    v, y, x, cls4 = v.ravel(), y.ravel(), x.ravel(), cls4.ravel()

    # exact 5x5 peak test (index clipping == -inf padding under max)
    d = np.arange(-2, 3)
    yy = np.clip(y[:, None] + d[None, :], 0, H - 1)
    xx = np.clip(x[:, None] + d[None, :], 0, W - 1)
    win = cls0[cls4[:, None, None], yy[:, :, None], xx[:, None, :]]
    keep = win.max(axis=(1, 2)) == v
    return v[keep], cls4[keep], y[keep], x[keep]


def _postprocess(cls0, txty0, twth0, cxall):
    flat = cxall.reshape(-1)
    ncand = 2048
    while True:
        if ncand >= flat.size:
            order = np.argsort(-flat, kind="stable")
            bound = -np.inf
        else:
            part = np.argpartition(-flat, ncand)[:ncand]
            order = part[np.argsort(-flat[part], kind="stable")]
            bound = flat[order[-1]]  # unexamined block maxima are <= bound
        v, cc, yy, xx = _verify_blocks(cls0, order)
        if v.size >= TOPK:
            # reference tie-break: lax.top_k is stable over index order, so
            # equal scores order by ascending class, then spatial position
            top = np.lexsort((yy * W + xx, cc, -v))[:TOPK]
            if bound < v[top[-1]]:
                break
        if ncand >= flat.size:
            if v.size < TOPK:  # fewer than 100 peaks exist; not reachable
                return _reference_numpy(cls0, txty0, twth0)
            break  # full scan: exact by construction
        ncand *= 8
    v, cc, yy, xx = v[top], cc[top], yy[top], xx[top]

    scores = (1.0 / (1.0 + np.exp(-v))).astype(np.float32)
    clses = cc.astype(np.int32)

    sig = lambda a: (1.0 / (1.0 + np.exp(-a.astype(np.float32)))).astype(np.float32)
    cxp = (sig(txty0[0, yy, xx]) + xx.astype(np.float32)) * STRIDE
    cyp = (sig(txty0[1, yy, xx]) + yy.astype(np.float32)) * STRIDE
    wp = np.exp(twth0[0, yy, xx].astype(np.float32)) * STRIDE
    hp = np.exp(twth0[1, yy, xx].astype(np.float32)) * STRIDE
    scale = np.float32(H * STRIDE)
    bbox = (
        np.stack([cxp - wp * 0.5, cyp - hp * 0.5, cxp + wp * 0.5, cyp + hp * 0.5], -1)
        / scale
    )
    bbox = np.clip(bbox, 0.0, 1.0).astype(np.float32)
    return bbox, scores, clses


def _reference_numpy(cls0, txty0, twth0):
    """Exact host-only implementation (device-failure safety net)."""
    prob = (1.0 / (1.0 + np.exp(-cls0.astype(np.float64)))).astype(np.float32)
    pad = np.full((prob.shape[0], H + 4, W + 4), -np.inf, np.float32)
    pad[:, 2:-2, 2:-2] = prob
    hmax = prob.copy()
    for ddy in range(5):
        for ddx in range(5):
            np.maximum(hmax, pad[:, ddy : ddy + H, ddx : ddx + W], out=hmax)
    masked = prob * (hmax == prob)
    C = masked.shape[0]
    flat = masked.reshape(C, H * W)
    i1 = np.argsort(-flat, axis=1, kind="stable")[:, :TOPK]
    s1 = np.take_along_axis(flat, i1, axis=1)
    s1f = s1.reshape(-1)
    i2 = np.argsort(-s1f, kind="stable")[:TOPK]
    clses = (i2 // TOPK).astype(np.int32)
    inds = i1.reshape(-1)[i2]
    yy, xx = inds // W, inds % W
    scores = s1f[i2].astype(np.float32)
    sig = lambda a: (1.0 / (1.0 + np.exp(-a.astype(np.float32)))).astype(np.float32)
    cxp = (sig(txty0[0, yy, xx]) + xx.astype(np.float32)) * STRIDE
    cyp = (sig(txty0[1, yy, xx]) + yy.astype(np.float32)) * STRIDE
    wp = np.exp(twth0[0, yy, xx].astype(np.float32)) * STRIDE
    hp = np.exp(twth0[1, yy, xx].astype(np.float32)) * STRIDE
    scale = np.float32(H * STRIDE)
    bbox = (
        np.stack([cxp - wp * 0.5, cyp - hp * 0.5, cxp + wp * 0.5, cyp + hp * 0.5], -1)
        / scale
    )
    return np.clip(bbox, 0.0, 1.0).astype(np.float32), scores, clses


def kernel(cls_pred, txty_pred, twth_pred):
    cls0 = np.ascontiguousarray(np.asarray(cls_pred[0], dtype=np.float32))
    txty0 = np.asarray(txty_pred[0], dtype=np.float32)
    twth0 = np.asarray(twth_pred[0], dtype=np.float32)
    # the axon-tunneled device occasionally reports NRT_EXEC_UNIT_UNRECOVERABLE
    # if a previous process tore down mid-flight; a backend reset + retry
    # recovers it, and the exact host fallback guarantees a correct answer
    for attempt in range(3):
        try:
            cxall, _ = _run_device(cls0)
            return _postprocess(cls0, txty0, twth0, cxall)
        except Exception:  # pragma: no cover - device-flake path
            if attempt == 2:
                break
            import time

            time.sleep(2.0)
            try:
                import jax.extend.backend

                jax.extend.backend.clear_backends()
            except Exception:
                pass
    return _reference_numpy(cls0, txty0, twth0)
